# revision 26
# baseline (speedup 1.0000x reference)
"""DGCNN forward on 8 Trainium2 NeuronCores.

Data-parallel: batch 16 is sharded 2-per-core; all weights replicated.
Per core, per point cloud the kernel computes, fully on device:
  - pairwise selection matrix d[i,j] = x_i.x_j - ||x_j||^2/2 - ||x_i||^2/2
    = -||x_i - x_j||^2/2 (same top-k as the reference; centered so the row
    max is 0, which keeps float quantization fine near the top ranks)
  - top-20 neighbor indices per point via mantissa-packed max8/match_replace:
    low 10 mantissa bits of d are replaced by the column index j, so the
    DVE max8 sort carries indices along (quantization 2^-13 relative, far
    below the top-20/21 gap scale)
  - edge conv via u/v decomposition:
      y = max_k lrelu(a*(W1 x_j + (W2-W1) x_i) + b)
        = lrelu(max_k(u_j) + v_i),  u = (aW1)x, v = (a(W2-W1))x + b
    (valid since lrelu is monotone and per-channel BN scale folds into W)
  - neighbor gather of u via GPSIMD indirect_copy, max over k on DVE
  - conv5 + global max/mean pool + 3-layer FC head (BN folded into weights)
"""

import numpy as np

N = 1024
K = 20
NCORES = 8
BPC = 2
EPS = 1e-5

TRACE = False          # test.py flips this to get a profile + exec time
LAST_RESULTS = {}      # stash for test.py (exec_time_ns etc.)
_CACHE = {}


# ---------------------------------------------------------------- host prep

def _f32(a):
    return np.ascontiguousarray(np.asarray(a, dtype=np.float32))


def _fold_bn(p):
    g, b, m, v = (np.asarray(t, np.float32) for t in p)
    a = (g / np.sqrt(v + EPS)).astype(np.float32)
    return a, (b - m * a).astype(np.float32)


class _CPack:
    """Packs many small f32 constants into one [128, CW] tensor."""

    def __init__(self):
        self.cols = []
        self.off = {}
        self.cur = 0

    def add(self, name, arr2d):
        arr2d = _f32(arr2d)
        assert arr2d.ndim == 2 and arr2d.shape[0] <= 128
        pad = np.zeros((128, arr2d.shape[1]), np.float32)
        pad[: arr2d.shape[0]] = arr2d
        self.cols.append(pad)
        self.off[name] = self.cur
        self.cur += arr2d.shape[1]

    def tensor(self):
        return np.concatenate(self.cols, axis=1)


def _host_prep(w1, w2, w3, w4, w5, bn, W6, W7, b7, W8, b8):
    cp = _CPack()
    ws = [_f32(w1), _f32(w2), _f32(w3), _f32(w4)]
    Cs = [3, 64, 64, 128]
    Os = [64, 64, 128, 256]
    for li in range(4):
        a, bb = _fold_bn(bn[f"bn{li + 1}"])
        W = ws[li]
        C, O = Cs[li], Os[li]
        W1p = W[:, :C] * a[:, None]
        W2m1 = (W[:, C:] - W[:, :C]) * a[:, None]
        wu = np.zeros((128, O), np.float32)
        wv = np.zeros((128, O), np.float32)
        if li < 3:  # stacked input: batch1 slice sits at partitions 64..64+C
            wu[0:C] = W1p.T
            wu[64 : 64 + C] = W1p.T
            wv[0:C] = W2m1.T
            wv[64 : 64 + C] = W2m1.T
        else:
            wu[0:C] = W1p.T
            wv[0:C] = W2m1.T
        cp.add(f"wu{li}", wu)
        cp.add(f"wv{li}", wv)
        cp.add(f"bv{li}", bb[None, :])
        if li < 2:
            bvc = np.tile(bb, 2)[:, None]
        elif li == 2:
            bvc = bb[:, None]
        else:
            bvc = bb.reshape(2, 128).T
        cp.add(f"bvc{li}", bvc)
    # emask columns: l1 b0/b1, l2&l3 b0/b1 (C=64), l4 (all-ones)
    em = np.zeros((128, 5), np.float32)
    em[0:3, 0] = 1.0
    em[64:67, 1] = 1.0
    em[0:64, 2] = 1.0
    em[64:128, 3] = 1.0
    em[:, 4] = 1.0
    cp.add("emask", em)
    cp.add("ones", np.ones((1, N), np.float32))
    a5, b5 = _fold_bn(bn["bn5"])
    lhsT5 = (_f32(w5) * a5[:, None]).T.copy()  # [512, 1024]
    cp.add("w5b", b5[None, :])
    a6, b6 = _fold_bn(bn["bn6"])
    lhsT6 = (_f32(W6) * a6[:, None]).T.copy()  # [2048, 512]
    lhsT6[1024:] *= 1.0 / N  # mean half consumes raw sums
    cp.add("w6b", b6[None, :])
    a7, b7bn = _fold_bn(bn["bn7"])
    lhsT7 = (_f32(W7) * a7[:, None]).T.copy()  # [512, 256]
    cp.add("w7b", (a7 * _f32(b7) + b7bn)[None, :])
    lhsT8 = _f32(W8).T.copy()  # [256, 40]
    w8c = np.concatenate([lhsT8[0:128], lhsT8[128:256]], axis=1)  # [128, 80]
    cp.add("w8", w8c)
    cp.add("w8b", _f32(b8)[None, :])

    cpacku = np.zeros((128, 4), np.uint32)
    cpacku[:, 0] = 0xFFFFFC00
    cpacku[:, 1] = 1023
    iota10 = np.broadcast_to(np.arange(N, dtype=np.uint32), (128, N)).copy()

    # conv5 segments: [seg, 128, 1024]; segs 0/1 duplicated across halves
    w5c = np.zeros((5, 128, N), np.float32)
    w5c[0, 0:64] = lhsT5[0:64]
    w5c[0, 64:128] = lhsT5[0:64]
    w5c[1, 0:64] = lhsT5[64:128]
    w5c[1, 64:128] = lhsT5[64:128]
    w5c[2] = lhsT5[128:256]
    w5c[3] = lhsT5[256:384]
    w5c[4] = lhsT5[384:512]
    w5c = w5c.transpose(1, 0, 2).copy()  # [128, 5, 1024]

    w6t = lhsT6.reshape(16, 128, 512).copy()  # [16, 128, 512]
    w7c = lhsT7.reshape(4, 128, 256).transpose(1, 0, 2).copy()  # [128, 4, 256]

    return dict(
        cpack=cp.tensor(),
        cpacku=cpacku,
        iota10=iota10,
        w5c=w5c,
        w6t=w6t,
        w7c=w7c,
    ), cp.off


# ---------------------------------------------------------------- device

def _build_module(CW, COFF):
    import concourse.bass as bass
    import concourse.bacc as bacc
    import concourse.mybir as mybir
    from concourse.tile import TileContext

    dt = mybir.dt
    f32, u32, u16 = dt.float32, dt.uint32, dt.uint16
    AX = mybir.AxisListType
    AL = mybir.AluOpType
    AF = mybir.ActivationFunctionType

    nc = bacc.Bacc(None)

    xin = nc.dram_tensor("xin", [2, 3, N], f32, kind="ExternalInput")
    cpack_d = nc.dram_tensor("cpack", [128, CW], f32, kind="ExternalInput")
    cpacku_d = nc.dram_tensor("cpacku", [128, 4], u32, kind="ExternalInput")
    iota_d = nc.dram_tensor("iota10", [128, N], u32, kind="ExternalInput")
    w5_d = nc.dram_tensor("w5c", [128, 5, N], f32, kind="ExternalInput")
    w6_d = nc.dram_tensor("w6t", [16, 128, 512], f32, kind="ExternalInput")
    w7_d = nc.dram_tensor("w7c", [128, 4, 256], f32, kind="ExternalInput")
    out_d = nc.dram_tensor("out", [2, 40], f32, kind="ExternalOutput")
    # idx buffers live in DRAM already wrapped for indirect_copy, split in
    # two blocks so each gather stays within the 1024-index ISA limit:
    #   A block (cols 0:1024): neighbor ranks k<16, col = (n//16)*16 + k
    #     -> 16 gathers of 64 points x 16 ranks each
    #   B block (cols 1024:1280): ranks 16..19, col = 1024 + (n//16)*4 + k-16
    #     -> 4 gathers of 256 points x 4 ranks each
    # Row p = n%16 (indirect_copy uses one index list per 16-partition group).
    idxb = [[nc.dram_tensor(f"idx_{b}_{l}", [16, 1280], u16) for l in range(4)]
            for b in range(2)]

    OFF = COFF

    with TileContext(nc) as tc:
        with (
            tc.tile_pool(name="consts", bufs=1) as cpool,
            tc.tile_pool(name="w1", bufs=1) as wp1,
            tc.tile_pool(name="w2", bufs=2) as wp2,
            tc.tile_pool(name="w3", bufs=3) as wp3,
            tc.tile_pool(name="psum", bufs=2, space="PSUM") as pp,
        ):
            cpk = cpool.tile([128, CW], f32, tag="cpk", name="cpk")
            nc.sync.dma_start(cpk[:], cpack_d[:])
            cpku = cpool.tile([128, 4], u32, tag="cpku", name="cpku")
            nc.sync.dma_start(cpku[:], cpacku_d[:])
            iota = cpool.tile([128, N], u32, tag="iota", name="iota")
            nc.sync.dma_start(iota[:], iota_d[:])
            w5c = cpool.tile([128, 5, N], f32, tag="w5c", name="w5c")
            nc.sync.dma_start(w5c[:], w5_d[:])
            w7c = cpool.tile([128, 4, 256], f32, tag="w7c", name="w7c")
            nc.sync.dma_start(w7c[:], w7_d[:])

            ones = cpk[0:1, OFF["ones"] : OFF["ones"] + N]

            # persistent activations
            xt1 = cpool.tile([128, N], f32, tag="xt1", name="xt1")
            xt2 = cpool.tile([128, N], f32, tag="xt2", name="xt2")
            xt3 = cpool.tile([128, N], f32, tag="xt3", name="xt3")
            xt4 = [cpool.tile([128, N], f32, tag=f"xt4_{b}", name=f"xt4_{b}") for b in range(2)]
            xt5 = [[cpool.tile([128, N], f32, tag=f"xt5_{b}_{o}", name=f"xt5_{b}_{o}") for o in range(2)]
                   for b in range(2)]

            nc.gpsimd.memset(xt1[:], 0.0)
            nc.sync.dma_start(xt1[0:3, :], xin[0, :, :])
            nc.sync.dma_start(xt1[64:67, :], xin[1, :, :])

            def emit_topk(dps, b, l, t):
                """dps: PSUM [128,1024] selection matrix for points
                t*128..t*128+128 of batch b; writes top-20 idx to DRAM."""
                dp = wp3.tile([128, N], u32, tag="dp", name="dp")
                inst = nc.vector.scalar_tensor_tensor(
                    out=dp[:], in0=dps[:].bitcast(u32), scalar=0xFFFFFC00,
                    in1=iota[:], op0=AL.bitwise_and, op1=AL.bitwise_or)
                inst.ins.ins[1].dtype = u32  # bitvec imm must be integer-typed
                dpf = dp[:].bitcast(f32)
                mv = wp3.tile([128, 24], f32, tag="mv", name="mv")
                nc.vector.max(mv[:, 0:8], dpf)
                nc.vector.match_replace(dpf, mv[:, 0:8], dpf, -1e30)
                nc.vector.max(mv[:, 8:16], dpf)
                nc.vector.match_replace(dpf, mv[:, 8:16], dpf, -1e30)
                nc.vector.max(mv[:, 16:24], dpf)
                ei = wp3.tile([128, 24], u32, tag="ei", name="ei")
                inst2 = nc.vector.tensor_scalar(
                    out=ei[:], in0=mv[:].bitcast(u32), scalar1=1023,
                    scalar2=None, op0=AL.bitwise_and)
                inst2.ins.ins[1].dtype = u32
                ei16 = wp3.tile([128, K], u16, tag="ei16", name="ei16")
                nc.vector.tensor_copy(
                    ei16[:], ei[:].bitcast(u16)[:, 0 : 2 * K : 2])
                dstA = idxb[b][l][:, 128 * t : 128 * t + 128].rearrange(
                    "p (nh k) -> nh p k", nh=8, k=16)
                nc.sync.dma_start(dstA, ei16[:, 0:16])
                dstB = idxb[b][l][:, 1024 + 32 * t : 1024 + 32 * t + 32].rearrange(
                    "p (nh k) -> nh p k", nh=8, k=4)
                nc.sync.dma_start(dstB, ei16[:, 16:20])

            def emit_widx(b, l, dst, groups, goff):
                """DMA wrapped idx_{b}_{l} [16,1280] into SBUF, replicated
                across `groups` 16-partition groups (one plain DMA each —
                partition-split/broadcast SBUF APs mis-lower)."""
                for g in range(groups):
                    gg = goff + g
                    nc.sync.dma_start(
                        dst[gg * 16 : (gg + 1) * 16, :], idxb[b][l][:])

            def emit_gather_max(ut, widx, gm):
                """gm[:, :] = max over the 20 gathered neighbors of ut."""
                gmB = wp2.tile([128, N], f32, tag="gmB", name="gmB")
                for cA in range(16):
                    gt = wp2.tile([128, 1024], f32, tag="gath", name="gath")
                    nc.gpsimd.indirect_copy(
                        gt[:], ut[:], widx[:, cA * 64 : (cA + 1) * 64], True)
                    nc.vector.tensor_reduce(
                        out=gm[:, cA * 64 : (cA + 1) * 64],
                        in_=gt[:].rearrange("p (nh k q) -> p nh q k",
                                            nh=4, k=16, q=16),
                        axis=AX.X, op=AL.max)
                for cB in range(4):
                    gt = wp2.tile([128, 1024], f32, tag="gath", name="gath")
                    nc.gpsimd.indirect_copy(
                        gt[:], ut[:],
                        widx[:, 1024 + cB * 64 : 1024 + (cB + 1) * 64], True)
                    nc.vector.tensor_reduce(
                        out=gmB[:, cB * 256 : (cB + 1) * 256],
                        in_=gt[:].rearrange("p (nh k q) -> p nh q k",
                                            nh=16, k=4, q=16),
                        axis=AX.X, op=AL.max)
                nc.vector.tensor_max(gm[:], gm[:], gmB[:])

            def matmul_d(xs, base, C, t, hsq):
                dps = pp.tile([128, N], f32, tag="d", name="d")
                for nb in range(2):
                    sl = slice(nb * 512, (nb + 1) * 512)
                    nc.tensor.matmul(
                        dps[:, sl],
                        lhsT=xs[base : base + C, t * 128 : (t + 1) * 128],
                        rhs=xs[base : base + C, sl], start=True, stop=False)
                    nc.tensor.matmul(
                        dps[:, sl], lhsT=ones[:, 0:128], rhs=hsq[0:1, sl],
                        start=False, stop=False)
                    nc.tensor.matmul(
                        dps[:, sl], lhsT=hsq[0:1, t * 128 : (t + 1) * 128],
                        rhs=ones[:, sl], start=False, stop=True)
                return dps

            def emit_sq(x2t, emcol, hsq):
                """hsq[0:1,:] = -0.5 * sum_c x^2 (masked partition sum)."""
                sqp = pp.tile([1, N], f32, tag="mm", name="mm")
                for nb in range(2):
                    sl = slice(nb * 512, (nb + 1) * 512)
                    nc.tensor.matmul(
                        sqp[0:1, sl],
                        lhsT=cpk[:, OFF["emask"] + emcol : OFF["emask"] + emcol + 1],
                        rhs=x2t[:, sl], start=True, stop=True)
                nc.scalar.mul(hsq[0:1, :], sqp[0:1, :], -0.5)

            def lrelu_into(dst, gm, vt):
                nc.gpsimd.tensor_add(gm[:], gm[:], vt[:])
                nc.vector.scalar_tensor_tensor(
                    out=dst, in0=gm[:], scalar=0.2, in1=gm[:],
                    op0=AL.mult, op1=AL.max)

            # ---------------- edge conv layers ----------------
            for l in range(4):
                stacked = l < 3
                C = [3, 64, 64, 128][l]
                O = [64, 64, 128, 256][l]
                xs_l = [xt1, xt2, xt3, None][l]
                wuo, wvo, bvo = OFF[f"wu{l}"], OFF[f"wv{l}"], OFF[f"bv{l}"]

                if stacked:
                    x2t = wp2.tile([128, N], f32, tag="scr", name="scr")
                    nc.scalar.square(x2t[:], xs_l[:])
                    hsqs = []
                    for b in range(2):
                        hsq = wp2.tile([1, N], f32, tag="sq", name="sq")
                        emcol = [0, 2, 2][l] + b
                        emit_sq(x2t, emcol, hsq)
                        hsqs.append(hsq)
                    # u, v (stacked out for l<2; per-batch psum for l==2)
                    bvc = OFF[f"bvc{l}"]
                    if l < 2:
                        up = pp.tile([128, N], f32, tag="mm", name="mm")
                        vp = pp.tile([128, N], f32, tag="mm", name="mm")
                        for b in range(2):
                            ba = 64 * b
                            for nb in range(2):
                                sl = slice(nb * 512, (nb + 1) * 512)
                                nc.tensor.matmul(
                                    up[ba : ba + 64, sl],
                                    lhsT=cpk[ba : ba + C, wuo : wuo + 64],
                                    rhs=xs_l[ba : ba + C, sl],
                                    start=True, stop=True)
                                nc.tensor.matmul(
                                    vp[ba : ba + 64, sl],
                                    lhsT=cpk[ba : ba + C, wvo : wvo + 64],
                                    rhs=xs_l[ba : ba + C, sl],
                                    start=True, stop=True)
                        ut = wp2.tile([128, N], f32, tag="u", name="u")
                        vt = wp2.tile([128, N], f32, tag="v", name="v")
                        nc.scalar.copy(ut[:], up[:])
                        nc.scalar.activation(vt[:], vp[:], AF.Identity,
                                             bias=cpk[:, bvc : bvc + 1], scale=1.0)
                        uts = {0: ut, 1: ut}
                        vts = {0: vt, 1: vt}
                    else:
                        uts, vts = {}, {}
                        for b in range(2):
                            ba = 64 * b
                            up = pp.tile([128, N], f32, tag="mm", name="mm")
                            vp = pp.tile([128, N], f32, tag="mm", name="mm")
                            for nb in range(2):
                                sl = slice(nb * 512, (nb + 1) * 512)
                                nc.tensor.matmul(
                                    up[:, sl],
                                    lhsT=cpk[ba : ba + C, wuo : wuo + 128],
                                    rhs=xs_l[ba : ba + C, sl],
                                    start=True, stop=True)
                                nc.tensor.matmul(
                                    vp[:, sl],
                                    lhsT=cpk[ba : ba + C, wvo : wvo + 128],
                                    rhs=xs_l[ba : ba + C, sl],
                                    start=True, stop=True)
                            ut = wp2.tile([128, N], f32, tag="u", name="u")
                            vt = wp2.tile([128, N], f32, tag="v", name="v")
                            nc.scalar.copy(ut[:], up[:])
                            nc.scalar.activation(vt[:], vp[:], AF.Identity,
                                                 bias=cpk[:, bvc : bvc + 1],
                                                 scale=1.0)
                            uts[b] = ut
                            vts[b] = vt
                    # top-k per batch
                    for b in range(2):
                        ba = 64 * b
                        for t in range(8):
                            dps = matmul_d(xs_l, ba, C, t, hsqs[b])
                            emit_topk(dps, b, l, t)
                else:
                    bvc = OFF[f"bvc{l}"]
                    uts, vts, hsqs = {}, {}, []
                    for b in range(2):
                        xsb = xt4[b]
                        x2t = wp2.tile([128, N], f32, tag="scr", name="scr")
                        nc.scalar.square(x2t[:], xsb[:])
                        hsq = wp2.tile([1, N], f32, tag="sq", name="sq")
                        emit_sq(x2t, 4, hsq)
                        hsqs.append(hsq)
                        for oc in range(2):
                            up = pp.tile([128, N], f32, tag="mm", name="mm")
                            vp = pp.tile([128, N], f32, tag="mm", name="mm")
                            for nb in range(2):
                                sl = slice(nb * 512, (nb + 1) * 512)
                                nc.tensor.matmul(
                                    up[:, sl],
                                    lhsT=cpk[0:128, wuo + oc * 128 : wuo + (oc + 1) * 128],
                                    rhs=xsb[:, sl], start=True, stop=True)
                                nc.tensor.matmul(
                                    vp[:, sl],
                                    lhsT=cpk[0:128, wvo + oc * 128 : wvo + (oc + 1) * 128],
                                    rhs=xsb[:, sl], start=True, stop=True)
                            ut = wp2.tile([128, N], f32, tag="u", name="u")
                            vt = wp2.tile([128, N], f32, tag="v", name="v")
                            nc.scalar.copy(ut[:], up[:])
                            nc.scalar.activation(vt[:], vp[:], AF.Identity,
                                                 bias=cpk[:, bvc + oc : bvc + oc + 1],
                                                 scale=1.0)
                            uts[(b, oc)] = ut
                            vts[(b, oc)] = vt
                        for t in range(8):
                            dps = matmul_d(xsb, 0, C, t, hsqs[b])
                            emit_topk(dps, b, l, t)

                # gather + K-max + lrelu
                if l < 2:
                    widx = wp1.tile([128, 1280], u16, tag="widx", name="widx")
                    emit_widx(0, l, widx, 4, 0)
                    emit_widx(1, l, widx, 4, 4)
                    gm = wp2.tile([128, N], f32, tag="gm", name="gm")
                    emit_gather_max(uts[0], widx, gm)
                    dst = [xt2, xt3][l]
                    lrelu_into(dst[:], gm, vts[0])
                elif l == 2:
                    for b in range(2):
                        widx = wp1.tile([128, 1280], u16, tag="widx", name="widx")
                        emit_widx(b, l, widx, 8, 0)
                        gm = wp2.tile([128, N], f32, tag="gm", name="gm")
                        emit_gather_max(uts[b], widx, gm)
                        lrelu_into(xt4[b][:], gm, vts[b])
                else:
                    for b in range(2):
                        widx = wp1.tile([128, 1280], u16, tag="widx", name="widx")
                        emit_widx(b, l, widx, 8, 0)
                        for oc in range(2):
                            gm = wp2.tile([128, N], f32, tag="gm", name="gm")
                            emit_gather_max(uts[(b, oc)], widx, gm)
                            lrelu_into(xt5[b][oc][:], gm, vts[(b, oc)])

            # ---------------- conv5 + pooling ----------------
            gboth = wp1.tile([128, 16, 2], f32, tag="gboth", name="gboth")
            for b in range(2):
                ba = 64 * b
                for ot in range(8):
                    oc = slice(ot * 128, (ot + 1) * 128)
                    hp = pp.tile([128, N], f32, tag="mm", name="mm")
                    for nb in range(2):
                        sl = slice(nb * 512, (nb + 1) * 512)
                        nc.tensor.matmul(hp[:, sl], lhsT=w5c[ba : ba + 64, 0, oc],
                                         rhs=xt2[ba : ba + 64, sl],
                                         start=True, stop=False)
                        nc.tensor.matmul(hp[:, sl], lhsT=w5c[ba : ba + 64, 1, oc],
                                         rhs=xt3[ba : ba + 64, sl],
                                         start=False, stop=False)
                        nc.tensor.matmul(hp[:, sl], lhsT=w5c[:, 2, oc],
                                         rhs=xt4[b][:, sl],
                                         start=False, stop=False)
                        nc.tensor.matmul(hp[:, sl], lhsT=w5c[:, 3, oc],
                                         rhs=xt5[b][0][:, sl],
                                         start=False, stop=False)
                        nc.tensor.matmul(hp[:, sl], lhsT=w5c[:, 4, oc],
                                         rhs=xt5[b][1][:, sl],
                                         start=False, stop=False)
                        nc.tensor.matmul(
                            hp[:, sl],
                            lhsT=cpk[0:1, OFF["w5b"] + ot * 128 : OFF["w5b"] + (ot + 1) * 128],
                            rhs=ones[:, sl], start=False, stop=True)
                    habs = wp2.tile([128, N], f32, tag="scr", name="scr")
                    nc.scalar.activation(habs[:], hp[:], AF.Abs, bias=0.0, scale=0.4)
                    nc.vector.scalar_tensor_tensor(
                        out=habs[:], in0=hp[:], scalar=0.6, in1=habs[:],
                        op0=AL.mult, op1=AL.add,
                        accum_out=gboth[:, 8 + ot, b : b + 1])
                    nc.vector.tensor_reduce(
                        out=gboth[:, ot, b : b + 1], in_=habs[:],
                        axis=AX.X, op=AL.max)

            # ---------------- FC head (both batches at once) ----------------
            h6ps = [pp.tile([128, 2], f32, tag=["d", "d", "mm", "mm"][oc],
                            name=f"fcp{oc}")
                    for oc in range(4)]
            for c in range(16):
                w6s = wp2.tile([128, 512], f32, tag="w6s", name="w6s")
                nc.sync.dma_start(w6s[:], w6_d[c, :, :])
                for oc in range(4):
                    nc.tensor.matmul(
                        h6ps[oc][:],
                        lhsT=w6s[:, oc * 128 : (oc + 1) * 128],
                        rhs=gboth[:, c, :], start=(c == 0), stop=False)
            for oc in range(4):
                nc.tensor.matmul(
                    h6ps[oc][:],
                    lhsT=cpk[0:1, OFF["w6b"] + oc * 128 : OFF["w6b"] + (oc + 1) * 128],
                    rhs=ones[:, 0:2], start=False, stop=True)
            fc1 = wp1.tile([128, 8], f32, tag="fc1", name="fc1")
            fc1b = wp1.tile([128, 8], f32, tag="fc1b", name="fc1b")
            for oc in range(4):
                sl = slice(oc * 2, (oc + 1) * 2)
                nc.scalar.activation(fc1[:, sl], h6ps[oc][:], AF.Abs,
                                     bias=0.0, scale=0.4)
                nc.vector.scalar_tensor_tensor(
                    out=fc1b[:, sl], in0=h6ps[oc][:], scalar=0.6,
                    in1=fc1[:, sl], op0=AL.mult, op1=AL.add)

            h7ps = [pp.tile([128, 2], f32, tag="d", name=f"fcq{oc}")
                    for oc in range(2)]
            for oc in range(2):
                for c4 in range(4):
                    nc.tensor.matmul(
                        h7ps[oc][:],
                        lhsT=w7c[:, c4, oc * 128 : (oc + 1) * 128],
                        rhs=fc1b[:, c4 * 2 : (c4 + 1) * 2],
                        start=(c4 == 0), stop=False)
                nc.tensor.matmul(
                    h7ps[oc][:],
                    lhsT=cpk[0:1, OFF["w7b"] + oc * 128 : OFF["w7b"] + (oc + 1) * 128],
                    rhs=ones[:, 0:2], start=False, stop=True)
            fc2 = wp1.tile([128, 4], f32, tag="fc2", name="fc2")
            fc2b = wp1.tile([128, 4], f32, tag="fc2b", name="fc2b")
            for oc in range(2):
                sl = slice(oc * 2, (oc + 1) * 2)
                nc.scalar.activation(fc2[:, sl], h7ps[oc][:], AF.Abs,
                                     bias=0.0, scale=0.4)
                nc.vector.scalar_tensor_tensor(
                    out=fc2b[:, sl], in0=h7ps[oc][:], scalar=0.6,
                    in1=fc2[:, sl], op0=AL.mult, op1=AL.add)

            outp = pp.tile([40, 2], f32, tag="mm", name="outp")
            for c2 in range(2):
                nc.tensor.matmul(
                    outp[:], lhsT=cpk[0:128, OFF["w8"] + c2 * 40 : OFF["w8"] + (c2 + 1) * 40],
                    rhs=fc2b[:, c2 * 2 : (c2 + 1) * 2],
                    start=(c2 == 0), stop=False)
            nc.tensor.matmul(
                outp[:], lhsT=cpk[0:1, OFF["w8b"] : OFF["w8b"] + 40],
                rhs=ones[:, 0:2], start=False, stop=True)
            osb = wp1.tile([40, 2], f32, tag="osb", name="osb")
            nc.scalar.copy(osb[:], outp[:])
            nc.sync.dma_start(out_d[0:1, :], osb[0:40, 0:1])
            nc.sync.dma_start(out_d[1:2, :], osb[0:40, 1:2])

    nc.compile()
    return nc


# ---------------------------------------------------------------- entry

def kernel(x, w1, w2, w3, w4, w5, bn, W6, W7, b7, W8, b8):
    from concourse import bass_utils

    consts, coff = _host_prep(w1, w2, w3, w4, w5, bn, W6, W7, b7, W8, b8)
    CW = consts["cpack"].shape[1]
    key = ("mod", CW)
    if key not in _CACHE:
        _CACHE[key] = _build_module(CW, coff)
    nc = _CACHE[key]

    x = _f32(x)
    in_maps = []
    for c in range(NCORES):
        m = {"xin": x[c * BPC : (c + 1) * BPC]}
        m.update(consts)
        in_maps.append(m)

    try:
        res = bass_utils.run_bass_kernel_spmd(
            nc, in_maps, core_ids=list(range(NCORES)), trace=TRACE)
    except ModuleNotFoundError:
        # no NTFF profiling hook in this container; run untraced
        res = bass_utils.run_bass_kernel_spmd(
            nc, in_maps, core_ids=list(range(NCORES)), trace=False)
    LAST_RESULTS["exec_time_ns"] = res.exec_time_ns
    LAST_RESULTS["profile_json"] = res.profile_json
    out = np.concatenate([r["out"] for r in res.results], axis=0)
    return out.astype(np.float32)
